# revision 2
# baseline (speedup 1.0000x reference)
"""GAT kernel v2: batched dma_gather + multi-engine pipeline.

Design (per core, dst-node sharding, edges grouped by dst-tile):
  ztab[l%2]: [Vp, 256] bf16 rows (z c-major 128 | ssrc 4 | sdst 4 | pad),
    rows stored in `perm` order so phase-1 writes are contiguous.
  Phase 1: 3-chunk groups: load hT [128,384] bf16, 3 matmuls -> PSUM,
    Act copy -> SBUF bf16, one SP DMA -> ztab rows (perm layout).
  Phase 2 per dst-tile: two dma_gather (lo/hi src halves, int16 idx) pull
    all edge rows [128, NBt, 256]; per block build oh (ts-4x is_equal),
    PE-transpose -> ohT (Act copy to SBUF); sde matmuls; tile-batched
    esc/lrelu (DVE) + exp (Act); per group msg = ge*exp (stride-0 AP
    broadcast); per block agg/den PSUM-accum matmuls; finalize ELU ->
    hsh bf16 [hc, n] -> AllGather, or layer-3 logits.
"""
import numpy as np
import ml_dtypes
from contextlib import ExitStack

import concourse.bass as bass
import concourse.tile as tile
from concourse import bacc, mybir
from concourse.bass import IndirectOffsetOnAxis, AP

P = 128
IN_DIM = 128
HEADS = 4
COUT = 32
HC = HEADS * COUT            # 128
DZE = 256                    # ztab row elements (bf16) = 512B
NEG = 0.2
NLAYERS = 3
GB = 4                       # blocks per group
CHG = 3                      # phase-1 chunks per write group

# c-major permutation: position c*4+h <- original h*32+c
PERM_CM = np.array([h * COUT + c for c in range(COUT) for h in range(HEADS)],
                   np.int64)


def make_cfg(V, ncores, tiles_per_core, hsplit=32768):
    VSH = tiles_per_core * P
    Vp = ncores * VSH
    assert hsplit <= 32768 and Vp - hsplit <= 32768
    return dict(V=V, Vp=Vp, ncores=ncores, VSH=VSH, TILES=tiles_per_core,
                HSPLIT=hsplit)


def node_perm(Vp):
    """perm[v] = ztab row of node v (phase-1 contiguous write layout)."""
    perm = np.zeros(Vp, np.int64)
    nchunks = Vp // P
    for g0 in range(0, nchunks, CHG):
        J = min(CHG, nchunks - g0)
        base = g0 * P
        for j in range(J):
            for p in range(P):
                perm[base + j * P + p] = base + p * J + j
    assert np.array_equal(np.sort(perm), np.arange(Vp))
    return perm


def host_prep(cfg, x, edge_index, Ws, a_src, a_dst, head_w, head_b):
    V, Vp, NC, VSH, TILES = cfg["V"], cfg["Vp"], cfg["ncores"], cfg["VSH"], cfg["TILES"]
    HSPLIT = cfg["HSPLIT"]
    src = np.asarray(edge_index[0], np.int64)
    dst = np.asarray(edge_index[1], np.int64)

    perm = node_perm(Vp)
    psrc = perm[src]                       # gather row index per edge

    core_of = dst // VSH
    tile_of = (dst % VSH) // P

    # per (core, tile): lo/hi edge lists (by psrc < HSPLIT), sorted by psrc
    nblo = np.zeros((NC, TILES), np.int64)
    nbhi = np.zeros((NC, TILES), np.int64)
    buckets = {}
    for c in range(NC):
        mc = core_of == c
        s_c, d_c, t_c = psrc[mc], dst[mc], tile_of[mc]
        for t in range(TILES):
            mt = t_c == t
            s_t, d_t = s_c[mt], d_c[mt]
            lo_m = s_t < HSPLIT
            order_lo = np.argsort(s_t[lo_m], kind="stable")
            order_hi = np.argsort(s_t[~lo_m], kind="stable")
            lo_idx = s_t[lo_m][order_lo]
            lo_dl = (d_t[lo_m][order_lo] - (c * VSH + t * P))
            hi_idx = s_t[~lo_m][order_hi] - HSPLIT
            hi_dl = (d_t[~lo_m][order_hi] - (c * VSH + t * P))
            buckets[(c, t)] = (lo_idx, lo_dl, hi_idx, hi_dl)
            nblo[c, t] = -(-len(lo_idx) // P)
            nbhi[c, t] = -(-len(hi_idx) // P)

    NBLO = nblo.max(axis=0)               # shared across cores
    NBHI = nbhi.max(axis=0)
    NBT = NBLO + NBHI
    NB0 = np.concatenate([[0], np.cumsum(NBT)])[:-1]
    NBTOT = int(NBT.sum())
    NBMAX = int(NBT.max())

    idxT = np.zeros((NC, 16, NBTOT * 8), np.int16)
    dcol = np.full((NC, P, NBTOT), -1.0, np.float32)
    for c in range(NC):
        for t in range(TILES):
            lo_idx, lo_dl, hi_idx, hi_dl = buckets[(c, t)]
            for sec, (sidx, sdl, nb) in enumerate(
                    [(lo_idx, lo_dl, int(NBLO[t])),
                     (hi_idx, hi_dl, int(NBHI[t]))]):
                nslots = nb * P
                if nslots == 0:
                    continue
                ii = np.zeros(nslots, np.int16)
                dd = np.full(nslots, -1.0, np.float32)
                ii[:len(sidx)] = sidx.astype(np.int16)
                dd[:len(sdl)] = sdl.astype(np.float32)
                b0 = int(NB0[t]) + (int(NBLO[t]) if sec else 0)
                for b in range(nb):
                    dcol[c, :, b0 + b] = dd[b * P:(b + 1) * P]
                co = b0 * 8
                # idx i at [i%16, co + i//16]
                idxT[c, :, co:co + nb * 8] = ii.reshape(nb * 8, 16).T
    idxT = np.tile(idxT, (1, 8, 1))        # replicate to 128 partitions

    # tns: ztab rows of each tile's nodes (for sdst indirect gather)
    tns = np.zeros((NC, P, TILES), np.int32)
    for c in range(NC):
        for t in range(TILES):
            tns[c, :, t] = perm[c * VSH + t * P + np.arange(P)]

    # weights: Wext [L, IN, 136] bf16: z c-major | ssrc | sdst
    Wext = np.zeros((NLAYERS, IN_DIM, HC + 8), np.float32)
    for li in range(NLAYERS):
        W = np.asarray(Ws[li], np.float32)          # [HC, IN]
        Msl = np.zeros((HC, HEADS), np.float32)
        Mdl = np.zeros((HC, HEADS), np.float32)
        for h in range(HEADS):
            Msl[h * COUT:(h + 1) * COUT, h] = np.asarray(a_src[li])[h]
            Mdl[h * COUT:(h + 1) * COUT, h] = np.asarray(a_dst[li])[h]
        WT = W.T                                    # [IN, HC]
        if li > 0:
            WT = WT[PERM_CM, :]                     # input rows in c-major
        Wext[li, :, 0:HC] = WT[:, PERM_CM]          # z cols c-major
        Wext[li, :, HC:HC + 4] = WT @ Msl
        Wext[li, :, HC + 4:HC + 8] = WT @ Mdl
    Wext = Wext.astype(np.float32)
    Wext_hi = Wext.astype(ml_dtypes.bfloat16).astype(np.float32)
    Wext_lo = Wext - Wext_hi

    xT = np.zeros((IN_DIM, Vp), np.float32)
    xT[:, :V] = np.asarray(x, np.float32).T
    xTb = xT.astype(ml_dtypes.bfloat16)
    xTlo = (xT - xTb.astype(np.float32)).astype(ml_dtypes.bfloat16)

    iotaF = np.tile(np.arange(P, dtype=np.float32)[None, :], (P, 1))
    eye = np.eye(P, dtype=np.float32)
    E4cf = np.zeros((4, P), np.float32)   # E4cf[h, c*4+h] = 1
    for c_ in range(COUT):
        for h in range(HEADS):
            E4cf[h, c_ * 4 + h] = 1.0
    hw = np.asarray(head_w, np.float32).reshape(HC)[PERM_CM].reshape(HC, 1)
    hb = float(np.asarray(head_b).reshape(-1)[0])

    assert NBMAX * 4 <= 252, f"NBMAX={NBMAX} exceeds PSUM sde budget"
    meta = dict(NBLO=[int(v) for v in NBLO], NBHI=[int(v) for v in NBHI],
                NB0=[int(v) for v in NB0], NBTOT=NBTOT, NBMAX=NBMAX, hb=hb)
    in_maps = []
    for c in range(NC):
        in_maps.append({
            "xTb": xTb, "xTlo": xTlo, "Wext": Wext_hi, "Wext_lo": Wext_lo,
            "E4cf": E4cf, "head_w": hw, "iotaF": iotaF, "eye": eye,
            "idxT": idxT[c], "dcol": dcol[c], "tns": tns[c],
        })
    return in_maps, meta


def build_nc(cfg, meta, repeat=1, debug_tile=None):
    V, Vp, NC, VSH, TILES = cfg["V"], cfg["Vp"], cfg["ncores"], cfg["VSH"], cfg["TILES"]
    HSPLIT = cfg["HSPLIT"]
    NBLO, NBHI, NB0 = meta["NBLO"], meta["NBHI"], meta["NB0"]
    NBTOT, NBMAX, hb = meta["NBTOT"], meta["NBMAX"], meta["hb"]
    NCH = Vp // P                      # phase-1 chunks

    nc = bacc.Bacc("TRN2", target_bir_lowering=False, debug=False, num_devices=NC)
    f32, bf16, i16, i32 = (mybir.dt.float32, mybir.dt.bfloat16,
                           mybir.dt.int16, mybir.dt.int32)

    xTbd = nc.dram_tensor("xTb", [IN_DIM, Vp], bf16, kind="ExternalInput").ap()
    xTld = nc.dram_tensor("xTlo", [IN_DIM, Vp], bf16, kind="ExternalInput").ap()
    Wextd = nc.dram_tensor("Wext", [NLAYERS, IN_DIM, HC + 8], f32,
                           kind="ExternalInput").ap()
    Wextld = nc.dram_tensor("Wext_lo", [NLAYERS, IN_DIM, HC + 8], f32,
                            kind="ExternalInput").ap()
    E4d = nc.dram_tensor("E4cf", [4, P], f32, kind="ExternalInput").ap()
    hwd = nc.dram_tensor("head_w", [HC, 1], f32, kind="ExternalInput").ap()
    iotd = nc.dram_tensor("iotaF", [P, P], f32, kind="ExternalInput").ap()
    eyed = nc.dram_tensor("eye", [P, P], f32, kind="ExternalInput").ap()
    idxd = nc.dram_tensor("idxT", [P, NBTOT * 8], i16, kind="ExternalInput").ap()
    dcold = nc.dram_tensor("dcol", [P, NBTOT], f32, kind="ExternalInput").ap()
    tnd = nc.dram_tensor("tns", [P, TILES], i32, kind="ExternalInput").ap()
    out = nc.dram_tensor("out", [VSH], f32, kind="ExternalOutput").ap()

    ztabs = [nc.dram_tensor(f"ztab{i}", [Vp, DZE], bf16) for i in range(2)]
    if debug_tile is not None:
        dbg_oh = nc.dram_tensor("dbg_oh", [P, NBMAX * P], f32,
                                kind="ExternalOutput").ap()
        dbg_sd = nc.dram_tensor("dbg_sd", [P, 8], f32,
                                kind="ExternalOutput").ap()
        dbg_esc = nc.dram_tensor("dbg_esc", [P, NBMAX * 4], f32,
                                 kind="ExternalOutput").ap()
        dbg_exp = nc.dram_tensor("dbg_exp", [P, NBMAX * 4], f32,
                                 kind="ExternalOutput").ap()
        dbg_xn = nc.dram_tensor("dbg_xn", [P, P], f32,
                                kind="ExternalOutput").ap()
        dbg_ge = nc.dram_tensor("dbg_ge", [P, NBMAX * DZE], f32,
                                kind="ExternalOutput").ap()
        dbg_agg = nc.dram_tensor("dbg_agg", [P, P], f32,
                                 kind="ExternalOutput").ap()
        dbg_den = nc.dram_tensor("dbg_den", [P, 4], f32,
                                 kind="ExternalOutput").ap()
        dbg_msg = nc.dram_tensor("dbg_msg", [P, GB * P], f32,
                                 kind="ExternalOutput").ap()
    hsh = nc.dram_tensor("hsh", [IN_DIM, VSH], bf16)
    hfull = nc.dram_tensor("hfull", [NC, IN_DIM, VSH], bf16, addr_space="Shared")

    with tile.TileContext(nc) as tc, ExitStack() as ctx:
        cst = ctx.enter_context(tc.tile_pool(name="cst", bufs=1))
        sbh = ctx.enter_context(tc.tile_pool(name="sbh", bufs=3))   # hT chunks
        sbz = ctx.enter_context(tc.tile_pool(name="sbz", bufs=3))   # phase1 z out
        sbg = ctx.enter_context(tc.tile_pool(name="sbg", bufs=2))   # ge tiles
        sbo = ctx.enter_context(tc.tile_pool(name="sbo", bufs=2))   # ohS per tile
        sbw = ctx.enter_context(tc.tile_pool(name="sbw", bufs=4))   # small work
        sbm = ctx.enter_context(tc.tile_pool(name="sbm", bufs=3))   # msg / ohTg
        psZ = ctx.enter_context(tc.tile_pool(name="psZ", bufs=2, space="PSUM"))
        psO = ctx.enter_context(tc.tile_pool(name="psO", bufs=2, space="PSUM"))
        psB = ctx.enter_context(tc.tile_pool(name="psB", bufs=2, space="PSUM"))
        psR = ctx.enter_context(tc.tile_pool(name="psR", bufs=2, space="PSUM"))

        # ---- constants ----
        iotaFS = cst.tile([P, P], f32)
        nc.sync.dma_start(out=iotaFS[:], in_=iotd[:, :])
        iotaB = cst.tile([P, P], bf16)
        nc.vector.tensor_copy(out=iotaB[:], in_=iotaFS[:])
        eyeF = cst.tile([P, P], f32)
        nc.sync.dma_start(out=eyeF[:], in_=eyed[:, :])
        eyeB = cst.tile([P, P], bf16)
        nc.vector.tensor_copy(out=eyeB[:], in_=eyeF[:])
        E4S = cst.tile([4, P], f32)
        nc.sync.dma_start(out=E4S[:], in_=E4d[:, :])
        hwF = cst.tile([HC, 1], f32)
        nc.sync.dma_start(out=hwF[:], in_=hwd[:, :])
        hwS = cst.tile([HC, 1], bf16)
        nc.vector.tensor_copy(out=hwS[:], in_=hwF[:])
        WextF = cst.tile([IN_DIM, NLAYERS * (HC + 8)], f32)
        for li in range(NLAYERS):
            nc.sync.dma_start(out=WextF[:, li * (HC + 8):(li + 1) * (HC + 8)],
                              in_=Wextd[li, :, :])
        WextS = cst.tile([IN_DIM, NLAYERS * (HC + 8)], bf16)
        nc.vector.tensor_copy(out=WextS[:], in_=WextF[:])
        for li in range(NLAYERS):
            nc.sync.dma_start(out=WextF[:, li * (HC + 8):(li + 1) * (HC + 8)],
                              in_=Wextld[li, :, :])
        WextLoS = cst.tile([IN_DIM, NLAYERS * (HC + 8)], bf16)
        nc.vector.tensor_copy(out=WextLoS[:], in_=WextF[:])
        idxS = cst.tile([P, NBTOT * 8], i16)
        nc.sync.dma_start(out=idxS[:], in_=idxd[:, :])
        dcolF = cst.tile([P, NBTOT], f32)
        nc.sync.dma_start(out=dcolF[:], in_=dcold[:, :])
        tnsS = cst.tile([P, TILES], i32)
        nc.sync.dma_start(out=tnsS[:], in_=tnd[:, :])
        logitS = cst.tile([1, VSH], f32)
        zsbs = []
        for zi in range(3):
            zt = cst.tile([P, CHG * DZE], bf16, name=f"zsb{zi}")
            nc.vector.memset(zt[:], 0.0)
            zsbs.append(zt)

        Exp = mybir.ActivationFunctionType.Exp
        Copy = mybir.ActivationFunctionType.Copy

        for _rep in range(repeat):
         for li in range(NLAYERS):
            WextL = WextS[:, li * (HC + 8):(li + 1) * (HC + 8)]
            WextLo = WextLoS[:, li * (HC + 8):(li + 1) * (HC + 8)]
            ztab = ztabs[li % 2]
            # ================= Phase 1 =================
            for g0 in range(0, NCH, CHG):
                J = min(CHG, NCH - g0)
                n0 = g0 * P
                hT = sbh.tile([IN_DIM, CHG * P], bf16, tag="hT")
                hTlo = None
                if li == 0:
                    nc.sync.dma_start(out=hT[:, :J * P],
                                      in_=xTbd[:, n0:n0 + J * P])
                    hTlo = sbh.tile([IN_DIM, CHG * P], bf16, tag="hTlo")
                    nc.sync.dma_start(out=hTlo[:, :J * P],
                                      in_=xTld[:, n0:n0 + J * P])
                else:
                    done = 0
                    while done < J * P:
                        g = n0 + done
                        blk, col = g // VSH, g % VSH
                        w = min(J * P - done, VSH - col)
                        nc.sync.dma_start(out=hT[:, done:done + w],
                                          in_=hfull.ap()[blk, :, col:col + w])
                        done += w
                zps = psZ.tile([P, CHG * (HC + 8)], f32, space="PSUM", tag="zps")
                for j in range(J):
                    sl = zps[:, j * (HC + 8):(j + 1) * (HC + 8)]
                    nc.tensor.matmul(out=sl, lhsT=hT[:, j * P:(j + 1) * P],
                                     rhs=WextL, start=True, stop=False,
                                     skip_group_check=True)
                    nc.tensor.matmul(out=sl, lhsT=hT[:, j * P:(j + 1) * P],
                                     rhs=WextLo, start=False, stop=hTlo is None,
                                     skip_group_check=True)
                    if hTlo is not None:
                        nc.tensor.matmul(out=sl,
                                         lhsT=hTlo[:, j * P:(j + 1) * P],
                                         rhs=WextL, start=False, stop=True,
                                         skip_group_check=True)
                zsb = zsbs[(g0 // CHG) % 3]
                zpsR = zps.rearrange("p (j c) -> p j c", c=HC + 8)
                # z part -> bf16 cols 0..127 of each row
                zsbB = zsb.rearrange("p (j c) -> p j c", c=DZE)
                nc.scalar.activation(out=zsbB[:, :J, 0:HC],
                                     in_=zpsR[:, :J, 0:HC], func=Copy)
                # scores -> f32 at bytes 256..287 (f32 elements 64..71)
                zsbF = zsb.bitcast(f32).rearrange("p (j c) -> p j c",
                                                  c=DZE // 2)
                nc.scalar.activation(out=zsbF[:, :J, 64:72],
                                     in_=zpsR[:, :J, HC:HC + 8], func=Copy)
                # rows n0 + p*J + j  <->  sbuf partition p, cols j*DZE..
                dst_ap = AP(ztab, n0 * DZE, [[DZE, P * J], [1, DZE]])
                nc.sync.dma_start(out=dst_ap, in_=zsb[:, :J * DZE])

            # ================= Phase 2 =================
            for t in range(TILES):
                nbt = NBLO[t] + NBHI[t]
                if nbt == 0:
                    continue
                ngt = -(-nbt // GB)
                ge = sbg.tile([P, NBMAX, DZE], bf16, tag="ge")
                # gathers (lo from ztab[0:HSPLIT], hi from ztab[HSPLIT:])
                for sec, (nb, b0s) in enumerate([(NBLO[t], 0), (NBHI[t], NBLO[t])]):
                    if nb == 0:
                        continue
                    base = 0 if sec == 0 else HSPLIT
                    rows = HSPLIT if sec == 0 else Vp - HSPLIT
                    src_ap = AP(ztab, base * DZE, [[DZE, rows], [1, DZE]])
                    co = (NB0[t] + b0s) * 8
                    nc.gpsimd.dma_gather(
                        out_ap=ge[:, b0s:b0s + nb, :],
                        in_ap=src_ap,
                        idxs_ap=idxS[:, co:co + nb * 8],
                        num_idxs=nb * P, num_idxs_reg=nb * P,
                        elem_size=DZE, single_packet=False)
                # sdst rows (f32 bytes 272..287 = bf16 elems 136..143)
                sdstS = sbw.tile([P, 8], bf16, tag="sdstS")
                nc.gpsimd.indirect_dma_start(
                    out=sdstS[:], out_offset=None, in_=ztab.ap()[:, :],
                    in_offset=IndirectOffsetOnAxis(ap=tnsS[:, t:t + 1], axis=0),
                    element_offset=HC + 8)
                sdstF = sdstS.bitcast(f32)
                # --- sweep 1: oh, ohT, sde ---
                # AD bank: ONE accumulation group: aggT [:,0:128] + denTT
                # [:,128:132] (agg bb=0 starts it, den bb=last stops it).
                big = psB.tile([P, 512], f32, space="PSUM", tag="big")
                aggT = big[:, 0:P]
                denTT = big[:, P:P + 4]
                # SD bank: sequential producers, each consumed before the
                # next start pending-zeroes the bank.
                sdb = psR.tile([P, 512], f32, space="PSUM", tag="sdb")
                sdePS = sdb[:, 0:nbt * 4]
                ohS = sbo.tile([P, nbt * P], bf16, tag="ohS")
                for g in range(ngt):
                    nbl = min(GB, nbt - g * GB)
                    ohTps = psO.tile([P, GB * P], bf16, space="PSUM", tag="ohTps")
                    for b in range(nbl):
                        col = NB0[t] + g * GB + b
                        sl = (g * GB + b) * P
                        nc.vector.tensor_scalar(
                            out=ohS[:, sl:sl + P], in0=iotaB[:],
                            scalar1=dcolF[:, col:col + 1], scalar2=None,
                            op0=mybir.AluOpType.is_equal)
                        nc.tensor.transpose(out=ohTps[:, b * P:(b + 1) * P],
                                            in_=ohS[:, sl:sl + P],
                                            identity=eyeB[:])
                    ohTg = sbm.tile([P, GB * P], f32, tag="ohTg")
                    nc.scalar.activation(out=ohTg[:, :nbl * P],
                                         in_=ohTps[:, :nbl * P], func=Copy)
                    for b in range(nbl):
                        bb = g * GB + b
                        nc.tensor.matmul(out=sdb[:, bb * 4:(bb + 1) * 4],
                                         lhsT=ohTg[:, b * P:(b + 1) * P],
                                         rhs=sdstF, start=True, stop=True,
                                         skip_group_check=True)
                # --- tile-level score pipeline (f32) ---
                geF = ge.bitcast(f32)           # [128, NBMAX, 128] f32 view
                escb = sbw.tile([P, NBMAX * 4], f32, tag="escb")
                nc.vector.tensor_tensor(out=escb[:, :nbt * 4],
                                        in0=geF[:, :nbt, 64:68],
                                        in1=sdePS,
                                        op=mybir.AluOpType.add)
                esc2 = sbw.tile([P, NBMAX * 4], f32, tag="esc2")
                nc.vector.tensor_scalar(out=esc2[:, :nbt * 4],
                                        in0=escb[:, :nbt * 4], scalar1=NEG,
                                        scalar2=None, op0=mybir.AluOpType.mult)
                nc.vector.tensor_tensor(out=escb[:, :nbt * 4],
                                        in0=escb[:, :nbt * 4],
                                        in1=esc2[:, :nbt * 4],
                                        op=mybir.AluOpType.max)
                expf = sbw.tile([P, NBMAX * 4], bf16, tag="expf")
                nc.scalar.activation(out=expf[:, :nbt * 4],
                                     in_=escb[:, :nbt * 4], func=Exp)
                # --- sweep 2: msg, agg, den ---
                for g in range(ngt):
                    nbl = min(GB, nbt - g * GB)
                    msgS = sbm.tile([P, GB * P], bf16, tag="msgS")
                    z_ap = AP(ge.tensor, ge.offset + (g * GB) * DZE,
                              [ge.ap[0], [DZE, nbl], [1, HC]])
                    e_ap = AP(expf.tensor, expf.offset + (g * GB) * 4,
                              [expf.ap[0], [4, nbl], [0, COUT], [1, 4]])
                    nc.vector.tensor_tensor(out=msgS[:, :nbl * P], in0=z_ap,
                                            in1=e_ap, op=mybir.AluOpType.mult)
                    for b in range(nbl):
                        bb = g * GB + b
                        first = bb == 0
                        last = bb == nbt - 1
                        # agg+den share ONE psum accumulation group: agg's
                        # first matmul starts it, den's last matmul stops it.
                        nc.tensor.matmul(out=aggT, lhsT=msgS[:, b * P:(b + 1) * P],
                                         rhs=ohS[:, bb * P:(bb + 1) * P],
                                         start=first, stop=False,
                                         skip_group_check=True)
                        nc.tensor.matmul(out=denTT,
                                         lhsT=ohS[:, bb * P:(bb + 1) * P],
                                         rhs=expf[:, bb * 4:(bb + 1) * 4],
                                         start=False, stop=last,
                                         skip_group_check=True)
                # --- finalize ---
                rexPS = sdb[:, 256:256 + P]
                dsbT = sdb[0:4, 384:512]
                lg = sdb[0:1, 0:P]
                dsb = sbw.tile([P, 4], f32, tag="dsb")
                nc.vector.tensor_scalar(out=dsb[:], in0=denTT, scalar1=1e-9,
                                        scalar2=None, op0=mybir.AluOpType.add)
                nc.vector.reciprocal(out=dsb[:], in_=dsb[:])
                nc.tensor.transpose(out=dsbT, in_=dsb[:], identity=eyeF[:])
                dsbT2 = sbw.tile([4, P], f32, tag="dsbT2")
                nc.scalar.activation(out=dsbT2[:], in_=dsbT, func=Copy)
                nc.tensor.matmul(out=rexPS, lhsT=E4S[:], rhs=dsbT2[:],
                                 start=True, stop=True)
                rexS = sbw.tile([P, P], f32, tag="rexS")
                nc.scalar.activation(out=rexS[:], in_=rexPS, func=Copy)
                # last layer keeps f32 through the head dot (bf16 h there
                # costs ~0.4% directly on the logits)
                hdt = bf16 if li < NLAYERS - 1 else f32
                xn = sbw.tile([P, P], hdt, tag="xn")
                nc.vector.tensor_tensor(out=xn[:], in0=aggT, in1=rexS[:],
                                        op=mybir.AluOpType.mult)
                # ELU
                tmin = sbw.tile([P, P], hdt, tag="tmin")
                nc.vector.tensor_scalar(out=tmin[:], in0=xn[:], scalar1=0.0,
                                        scalar2=None, op0=mybir.AluOpType.min)
                texp = sbw.tile([P, P], hdt, tag="texp")
                nc.scalar.activation(out=texp[:], in_=tmin[:], func=Exp)
                trel = sbw.tile([P, P], hdt, tag="trel")
                nc.vector.tensor_scalar(out=trel[:], in0=xn[:], scalar1=0.0,
                                        scalar2=None, op0=mybir.AluOpType.max)
                hsum = sbw.tile([P, P], hdt, tag="hsum")
                nc.vector.tensor_tensor(out=hsum[:], in0=trel[:], in1=texp[:],
                                        op=mybir.AluOpType.add)
                hb16 = sbw.tile([P, P], hdt, tag="hb16")
                nc.vector.tensor_scalar(out=hb16[:], in0=hsum[:], scalar1=-1.0,
                                        scalar2=None, op0=mybir.AluOpType.add)
                if debug_tile is not None and li == 0 and t == debug_tile:
                    dbf = sbw.tile([P, NBMAX * DZE], f32, tag="dbf", bufs=1)
                    nc.vector.tensor_copy(out=dbf[:, :nbt * P], in_=ohS[:])
                    nc.sync.dma_start(out=dbg_oh[:, :nbt * P],
                                      in_=dbf[:, :nbt * P])
                    nc.vector.tensor_copy(out=dbf[:, :4], in_=sdstF)
                    nc.sync.dma_start(out=dbg_sd[:, :4], in_=dbf[:, :4])
                    nc.vector.tensor_copy(out=dbf[:, :nbt * 4],
                                          in_=escb[:, :nbt * 4])
                    nc.sync.dma_start(out=dbg_esc[:, :nbt * 4],
                                      in_=dbf[:, :nbt * 4])
                    nc.vector.tensor_copy(out=dbf[:, :nbt * 4],
                                          in_=expf[:, :nbt * 4])
                    nc.sync.dma_start(out=dbg_exp[:, :nbt * 4],
                                      in_=dbf[:, :nbt * 4])
                    nc.vector.tensor_copy(out=dbf[:, :P], in_=xn[:])
                    nc.sync.dma_start(out=dbg_xn[:, :P], in_=dbf[:, :P])
                    nc.vector.tensor_copy(out=dbf[:, :nbt * DZE],
                                          in_=ge[:, :nbt, :])
                    nc.sync.dma_start(out=dbg_ge[:, :nbt * DZE],
                                      in_=dbf[:, :nbt * DZE])
                    nc.vector.tensor_copy(out=dbf[:, :P], in_=aggT)
                    nc.sync.dma_start(out=dbg_agg[:, :], in_=dbf[:, :P])
                    nc.vector.tensor_copy(out=dbf[:, :4], in_=denTT)
                    nc.sync.dma_start(out=dbg_den[:, :], in_=dbf[:, :4])
                if li < NLAYERS - 1:
                    nc.sync.dma_start(out=hsh.ap()[:, t * P:(t + 1) * P],
                                      in_=hb16[:])
                else:
                    nc.tensor.matmul(out=lg, lhsT=hwF[:], rhs=hb16[:],
                                     start=True, stop=True)
                    nc.vector.tensor_scalar(out=logitS[:, t * P:(t + 1) * P],
                                            in0=lg, scalar1=hb, scalar2=None,
                                            op0=mybir.AluOpType.add)
            if li < NLAYERS - 1:
                nc.gpsimd.collective_compute(
                    "AllGather", mybir.AluOpType.bypass,
                    replica_groups=[list(range(NC))],
                    ins=[hsh.ap()[:, :]], outs=[hfull.ap()[:, :, :]])
        nc.sync.dma_start(out=out[None, :], in_=logitS[:])
    nc.compile()
    return nc


# ======================= runner =======================

import time
import numpy as np
import jax
from jax.sharding import Mesh, PartitionSpec
from jax.experimental.shard_map import shard_map

import concourse.mybir as mybir
from concourse import bass2jax
from concourse.bass2jax import _bass_exec_p, install_neuronx_cc_hook, partition_id_tensor


class SpmdRunner:
    def __init__(self, nc, n_cores: int):
        install_neuronx_cc_hook()
        assert nc.dbg_addr is None or not nc.dbg_callbacks
        self.nc = nc
        self.n_cores = n_cores
        partition_name = nc.partition_id_tensor.name if nc.partition_id_tensor else None

        in_names, out_names, out_avals, zero_outs = [], [], [], []
        for alloc in nc.m.functions[0].allocations:
            if not isinstance(alloc, mybir.MemoryLocationSet):
                continue
            name = alloc.memorylocations[0].name
            if alloc.kind == "ExternalInput":
                if name != partition_name and name != (nc.dbg_addr.name if nc.dbg_addr else None):
                    in_names.append(name)
            elif alloc.kind == "ExternalOutput":
                out_names.append(name)
                shape = tuple(alloc.tensor_shape)
                dtype = mybir.dt.np(alloc.dtype)
                out_avals.append(jax.core.ShapedArray(shape, dtype))
                zero_outs.append(np.zeros(shape, dtype))
        self.in_names, self.out_names = in_names, out_names
        self.out_avals, self.zero_outs = out_avals, zero_outs
        n_params = len(in_names)
        self.n_params = n_params
        n_outs = len(out_avals)

        all_in_names = list(in_names) + list(out_names)
        if nc.dbg_addr is not None:
            all_in_names.append(nc.dbg_addr.name)
        if partition_name is not None:
            all_in_names.append(partition_name)

        dbg_name = nc.dbg_addr.name if nc.dbg_addr is not None else None

        def _body(*args):
            operands = list(args)
            if dbg_name is not None:
                operands.append(np.zeros((1, 2), np.uint32))
            if partition_name is not None:
                operands.append(partition_id_tensor())
            outs = _bass_exec_p.bind(
                *operands,
                out_avals=tuple(out_avals),
                in_names=tuple(all_in_names),
                out_names=tuple(out_names),
                lowering_input_output_aliases=(),
                sim_require_finite=False,
                sim_require_nnan=False,
                nc=nc,
            )
            return tuple(outs)

        devices = jax.devices()[:n_cores]
        assert len(devices) == n_cores
        self.mesh = Mesh(np.asarray(devices), ("core",))
        in_specs = (PartitionSpec("core"),) * (n_params + n_outs)
        out_specs = (PartitionSpec("core"),) * n_outs
        self.donate = tuple(range(n_params, n_params + n_outs))
        self.fn = jax.jit(
            shard_map(_body, mesh=self.mesh, in_specs=in_specs,
                      out_specs=out_specs, check_rep=False),
            donate_argnums=self.donate, keep_unused=True,
        )
        self.concat_in = None

    def load_inputs(self, in_maps):
        """Concat per-core inputs and push to devices once."""
        assert len(in_maps) == self.n_cores
        per_core = [[np.asarray(m[name]) for name in self.in_names] for m in in_maps]
        concat = [np.concatenate([per_core[c][i] for c in range(self.n_cores)], axis=0)
                  for i in range(self.n_params)]
        sh = jax.sharding.NamedSharding(self.mesh, PartitionSpec("core"))
        self.concat_in = [jax.device_put(a, sh) for a in concat]

    def _zeros(self):
        sh = jax.sharding.NamedSharding(self.mesh, PartitionSpec("core"))
        return [jax.device_put(np.zeros((self.n_cores * z.shape[0], *z.shape[1:]), z.dtype), sh)
                for z in self.zero_outs]

    def run(self):
        outs = self.fn(*self.concat_in, *self._zeros())
        jax.block_until_ready(outs)
        return [
            {name: np.asarray(outs[i]).reshape(self.n_cores, *self.out_avals[i].shape)[c]
             for i, name in enumerate(self.out_names)}
            for c in range(self.n_cores)
        ]

    def time(self, iters=8, warmup=2):
        """Per-call wall time (s) for the jitted executable, zeros pre-staged."""
        zs = [self._zeros() for _ in range(iters + warmup)]
        for i in range(warmup):
            jax.block_until_ready(self.fn(*self.concat_in, *zs[i]))
        ts = []
        for i in range(iters):
            t0 = time.perf_counter()
            jax.block_until_ready(self.fn(*self.concat_in, *zs[warmup + i]))
            ts.append(time.perf_counter() - t0)
        return min(ts), ts



# ======================= driver =======================
import time as _time
import jax as _jax

_CACHE = {}
LAST_EXEC_NS = None


def _floor_nc(ncores):
    """Tiny kernel to estimate the per-call dispatch floor."""
    nc = bacc.Bacc("TRN2", target_bir_lowering=False, debug=False, num_devices=ncores)
    a = nc.dram_tensor("a", [P, 64], mybir.dt.float32, kind="ExternalInput").ap()
    b = nc.dram_tensor("b", [P, 64], mybir.dt.float32, kind="ExternalOutput").ap()
    with tile.TileContext(nc) as tc, ExitStack() as ctx:
        sb = ctx.enter_context(tc.tile_pool(name="sb", bufs=2))
        t = sb.tile([P, 64], mybir.dt.float32)
        nc.sync.dma_start(out=t[:], in_=a[:, :])
        nc.sync.dma_start(out=b[:, :], in_=t[:])
    nc.compile()
    return nc


def kernel(x, edge_index, Ws, a_src, a_dst, head_w, head_b, hsplit=32768,
           tiles_per_core=None, repeat=1):
    NC = 8
    V = int(np.asarray(x).shape[0])
    if tiles_per_core is None:
        tiles_per_core = (V + NC * P - 1) // (NC * P)
    cfg = make_cfg(V, NC, tiles_per_core, hsplit)
    in_maps, meta = host_prep(cfg, x, edge_index, Ws, a_src, a_dst,
                              head_w, head_b)
    key = (V, tuple(meta["NBLO"]), tuple(meta["NBHI"]))
    if key not in _CACHE:
        nc = build_nc(cfg, meta, repeat=repeat)
        r = SpmdRunner(nc, NC)
        _CACHE[key] = r
    r = _CACHE[key]
    r.load_inputs(in_maps)
    res = r.run()
    out = np.concatenate([res[c]["out"] for c in range(NC)])[:V]
    return out.astype(np.float32)


def measure(iters=16):
    """Estimate HW exec ns via interleaved kernel/floor timing."""
    global LAST_EXEC_NS
    assert _CACHE, "call kernel() first"
    r = next(iter(_CACHE.values()))
    fnc = _floor_nc(r.n_cores)
    fr = SpmdRunner(fnc, r.n_cores)
    fr.load_inputs([{"a": np.zeros((P, 64), np.float32)}] * r.n_cores)
    fr.run()
    r.run()
    diffs, ks, fs = [], [], []
    for _ in range(iters):
        z = r._zeros()
        t0 = _time.perf_counter()
        _jax.block_until_ready(r.fn(*r.concat_in, *z))
        tk = _time.perf_counter() - t0
        zf = fr._zeros()
        t0 = _time.perf_counter()
        _jax.block_until_ready(fr.fn(*fr.concat_in, *zf))
        tf = _time.perf_counter() - t0
        ks.append(tk); fs.append(tf); diffs.append(tk - tf)
    diffs.sort()
    med = diffs[len(diffs) // 2]
    LAST_EXEC_NS = int(max(0.0, med) * 1e9)
    return LAST_EXEC_NS, sorted(ks)[len(ks)//2], sorted(fs)[len(fs)//2]


def gat_reference_np(x, edge_index, Ws, a_src, a_dst, head_w, head_b):
    V = x.shape[0]
    src = np.asarray(edge_index[0]); dst = np.asarray(edge_index[1])
    h = np.asarray(x, np.float64)
    for li in range(len(Ws)):
        z = (h @ np.asarray(Ws[li], np.float64).T).reshape(V, HEADS, COUT)
        ss = np.einsum("vhc,hc->vh", z, np.asarray(a_src[li], np.float64))
        sd = np.einsum("vhc,hc->vh", z, np.asarray(a_dst[li], np.float64))
        e = ss[src] + sd[dst]
        e = np.where(e > 0, e, NEG * e)
        m = np.full((V, HEADS), -np.inf); np.maximum.at(m, dst, e)
        m = np.maximum(m, -1e9)
        ex = np.exp(e - m[dst])
        den = np.zeros((V, HEADS)); np.add.at(den, dst, ex)
        alpha = ex / (den[dst] + 1e-9)
        msg = z[src] * alpha[:, :, None]
        agg = np.zeros((V, HEADS, COUT)); np.add.at(agg, dst, msg)
        h = np.where(agg > 0, agg, np.expm1(agg)).reshape(V, HC)
    return (h @ np.asarray(head_w, np.float64).T + np.asarray(head_b)).reshape(V)


# revision 3
# speedup vs baseline: 1.6428x; 1.6428x over previous
"""GAT kernel v2: batched dma_gather + multi-engine pipeline.

Design (per core, dst-node sharding, edges grouped by dst-tile):
  ztab[l%2]: [Vp, 256] bf16 rows (z c-major 128 | ssrc 4 | sdst 4 | pad),
    rows stored in `perm` order so phase-1 writes are contiguous.
  Phase 1: 3-chunk groups: load hT [128,384] bf16, 3 matmuls -> PSUM,
    Act copy -> SBUF bf16, one SP DMA -> ztab rows (perm layout).
  Phase 2 per dst-tile: two dma_gather (lo/hi src halves, int16 idx) pull
    all edge rows [128, NBt, 256]; per block build oh (ts-4x is_equal),
    PE-transpose -> ohT (Act copy to SBUF); sde matmuls; tile-batched
    esc/lrelu (DVE) + exp (Act); per group msg = ge*exp (stride-0 AP
    broadcast); per block agg/den PSUM-accum matmuls; finalize ELU ->
    hsh bf16 [hc, n] -> AllGather, or layer-3 logits.
"""
import numpy as np
import ml_dtypes
from contextlib import ExitStack

import concourse.bass as bass
import concourse.tile as tile
from concourse import bacc, mybir
from concourse.bass import IndirectOffsetOnAxis, AP

P = 128
IN_DIM = 128
HEADS = 4
COUT = 32
HC = HEADS * COUT            # 128
DZE = 256                    # ztab row elements (bf16) = 512B
NEG = 0.2
NLAYERS = 3
GB = 4                       # blocks per group
CHG = 3                      # phase-1 chunks per write group

# c-major permutation: position c*4+h <- original h*32+c
PERM_CM = np.array([h * COUT + c for c in range(COUT) for h in range(HEADS)],
                   np.int64)


def make_cfg(V, ncores, tiles_per_core, hsplit=32768):
    VSH = tiles_per_core * P
    Vp = ncores * VSH
    assert hsplit <= 32768 and Vp - hsplit <= 32768
    return dict(V=V, Vp=Vp, ncores=ncores, VSH=VSH, TILES=tiles_per_core,
                HSPLIT=hsplit)


def node_perm(Vp):
    """perm[v] = ztab row of node v (phase-1 contiguous write layout)."""
    perm = np.zeros(Vp, np.int64)
    nchunks = Vp // P
    for g0 in range(0, nchunks, CHG):
        J = min(CHG, nchunks - g0)
        base = g0 * P
        for j in range(J):
            for p in range(P):
                perm[base + j * P + p] = base + p * J + j
    assert np.array_equal(np.sort(perm), np.arange(Vp))
    return perm


def host_prep(cfg, x, edge_index, Ws, a_src, a_dst, head_w, head_b):
    V, Vp, NC, VSH, TILES = cfg["V"], cfg["Vp"], cfg["ncores"], cfg["VSH"], cfg["TILES"]
    HSPLIT = cfg["HSPLIT"]
    src = np.asarray(edge_index[0], np.int64)
    dst = np.asarray(edge_index[1], np.int64)

    perm = node_perm(Vp)
    psrc = perm[src]                       # gather row index per edge

    core_of = dst // VSH
    tile_of = (dst % VSH) // P

    # per (core, tile): lo/hi edge lists (by psrc < HSPLIT), sorted by psrc
    nblo = np.zeros((NC, TILES), np.int64)
    nbhi = np.zeros((NC, TILES), np.int64)
    buckets = {}
    for c in range(NC):
        mc = core_of == c
        s_c, d_c, t_c = psrc[mc], dst[mc], tile_of[mc]
        for t in range(TILES):
            mt = t_c == t
            s_t, d_t = s_c[mt], d_c[mt]
            lo_m = s_t < HSPLIT
            order_lo = np.argsort(s_t[lo_m], kind="stable")
            order_hi = np.argsort(s_t[~lo_m], kind="stable")
            lo_idx = s_t[lo_m][order_lo]
            lo_dl = (d_t[lo_m][order_lo] - (c * VSH + t * P))
            hi_idx = s_t[~lo_m][order_hi] - HSPLIT
            hi_dl = (d_t[~lo_m][order_hi] - (c * VSH + t * P))
            buckets[(c, t)] = (lo_idx, lo_dl, hi_idx, hi_dl)
            nblo[c, t] = -(-len(lo_idx) // P)
            nbhi[c, t] = -(-len(hi_idx) // P)

    NBLO = nblo.max(axis=0)               # shared across cores
    NBHI = nbhi.max(axis=0)
    NBT = NBLO + NBHI
    NB0 = np.concatenate([[0], np.cumsum(NBT)])[:-1]
    NBTOT = int(NBT.sum())
    NBMAX = int(NBT.max())

    idxT = np.zeros((NC, 16, NBTOT * 8), np.int16)
    dcol = np.full((NC, P, NBTOT), -1.0, np.float32)
    for c in range(NC):
        for t in range(TILES):
            lo_idx, lo_dl, hi_idx, hi_dl = buckets[(c, t)]
            for sec, (sidx, sdl, nb) in enumerate(
                    [(lo_idx, lo_dl, int(NBLO[t])),
                     (hi_idx, hi_dl, int(NBHI[t]))]):
                nslots = nb * P
                if nslots == 0:
                    continue
                ii = np.zeros(nslots, np.int16)
                dd = np.full(nslots, -1.0, np.float32)
                ii[:len(sidx)] = sidx.astype(np.int16)
                dd[:len(sdl)] = sdl.astype(np.float32)
                b0 = int(NB0[t]) + (int(NBLO[t]) if sec else 0)
                for b in range(nb):
                    dcol[c, :, b0 + b] = dd[b * P:(b + 1) * P]
                co = b0 * 8
                # idx i at [i%16, co + i//16]
                idxT[c, :, co:co + nb * 8] = ii.reshape(nb * 8, 16).T
    idxT = np.tile(idxT, (1, 8, 1))        # replicate to 128 partitions

    # tns: ztab rows of each tile's nodes (for sdst indirect gather)
    tns = np.zeros((NC, P, TILES), np.int32)
    for c in range(NC):
        for t in range(TILES):
            tns[c, :, t] = perm[c * VSH + t * P + np.arange(P)]

    # weights: Wext [L, IN, 136] bf16: z c-major | ssrc | sdst
    Wext = np.zeros((NLAYERS, IN_DIM, HC + 8), np.float32)
    for li in range(NLAYERS):
        W = np.asarray(Ws[li], np.float32)          # [HC, IN]
        Msl = np.zeros((HC, HEADS), np.float32)
        Mdl = np.zeros((HC, HEADS), np.float32)
        for h in range(HEADS):
            Msl[h * COUT:(h + 1) * COUT, h] = np.asarray(a_src[li])[h]
            Mdl[h * COUT:(h + 1) * COUT, h] = np.asarray(a_dst[li])[h]
        WT = W.T                                    # [IN, HC]
        if li > 0:
            WT = WT[PERM_CM, :]                     # input rows in c-major
        Wext[li, :, 0:HC] = WT[:, PERM_CM]          # z cols c-major
        Wext[li, :, HC:HC + 4] = WT @ Msl
        Wext[li, :, HC + 4:HC + 8] = WT @ Mdl
    Wext = Wext.astype(np.float32)
    Wext_hi = Wext.astype(ml_dtypes.bfloat16).astype(np.float32)
    Wext_lo = Wext - Wext_hi

    xT = np.zeros((IN_DIM, Vp), np.float32)
    xT[:, :V] = np.asarray(x, np.float32).T
    xTb = xT.astype(ml_dtypes.bfloat16)
    xTlo = (xT - xTb.astype(np.float32)).astype(ml_dtypes.bfloat16)

    iotaF = np.tile(np.arange(P, dtype=np.float32)[None, :], (P, 1))
    eye = np.eye(P, dtype=np.float32)
    E4cf = np.zeros((4, P), np.float32)   # E4cf[h, c*4+h] = 1
    for c_ in range(COUT):
        for h in range(HEADS):
            E4cf[h, c_ * 4 + h] = 1.0
    hw = np.asarray(head_w, np.float32).reshape(HC)[PERM_CM].reshape(HC, 1)
    hb = float(np.asarray(head_b).reshape(-1)[0])

    assert NBMAX * 4 <= 252, f"NBMAX={NBMAX} exceeds PSUM sde budget"
    meta = dict(NBLO=[int(v) for v in NBLO], NBHI=[int(v) for v in NBHI],
                NB0=[int(v) for v in NB0], NBTOT=NBTOT, NBMAX=NBMAX, hb=hb)
    in_maps = []
    for c in range(NC):
        in_maps.append({
            "xTb": xTb, "xTlo": xTlo, "Wext": Wext_hi, "Wext_lo": Wext_lo,
            "E4cf": E4cf, "head_w": hw, "iotaF": iotaF, "eye": eye,
            "idxT": idxT[c], "dcol": dcol[c], "tns": tns[c],
        })
    return in_maps, meta


def build_nc(cfg, meta, repeat=1, debug_tile=None):
    V, Vp, NC, VSH, TILES = cfg["V"], cfg["Vp"], cfg["ncores"], cfg["VSH"], cfg["TILES"]
    HSPLIT = cfg["HSPLIT"]
    NBLO, NBHI, NB0 = meta["NBLO"], meta["NBHI"], meta["NB0"]
    NBTOT, NBMAX, hb = meta["NBTOT"], meta["NBMAX"], meta["hb"]
    NCH = Vp // P                      # phase-1 chunks

    nc = bacc.Bacc("TRN2", target_bir_lowering=False, debug=False, num_devices=NC)
    f32, bf16, i16, i32 = (mybir.dt.float32, mybir.dt.bfloat16,
                           mybir.dt.int16, mybir.dt.int32)

    xTbd = nc.dram_tensor("xTb", [IN_DIM, Vp], bf16, kind="ExternalInput").ap()
    xTld = nc.dram_tensor("xTlo", [IN_DIM, Vp], bf16, kind="ExternalInput").ap()
    Wextd = nc.dram_tensor("Wext", [NLAYERS, IN_DIM, HC + 8], f32,
                           kind="ExternalInput").ap()
    Wextld = nc.dram_tensor("Wext_lo", [NLAYERS, IN_DIM, HC + 8], f32,
                            kind="ExternalInput").ap()
    E4d = nc.dram_tensor("E4cf", [4, P], f32, kind="ExternalInput").ap()
    hwd = nc.dram_tensor("head_w", [HC, 1], f32, kind="ExternalInput").ap()
    iotd = nc.dram_tensor("iotaF", [P, P], f32, kind="ExternalInput").ap()
    eyed = nc.dram_tensor("eye", [P, P], f32, kind="ExternalInput").ap()
    idxd = nc.dram_tensor("idxT", [P, NBTOT * 8], i16, kind="ExternalInput").ap()
    dcold = nc.dram_tensor("dcol", [P, NBTOT], f32, kind="ExternalInput").ap()
    tnd = nc.dram_tensor("tns", [P, TILES], i32, kind="ExternalInput").ap()
    out = nc.dram_tensor("out", [VSH], f32, kind="ExternalOutput").ap()

    ztabs = [nc.dram_tensor(f"ztab{i}", [Vp, DZE], bf16) for i in range(2)]
    if debug_tile is not None:
        dbg_oh = nc.dram_tensor("dbg_oh", [P, NBMAX * P], f32,
                                kind="ExternalOutput").ap()
        dbg_sd = nc.dram_tensor("dbg_sd", [P, 8], f32,
                                kind="ExternalOutput").ap()
        dbg_esc = nc.dram_tensor("dbg_esc", [P, NBMAX * 4], f32,
                                 kind="ExternalOutput").ap()
        dbg_exp = nc.dram_tensor("dbg_exp", [P, NBMAX * 4], f32,
                                 kind="ExternalOutput").ap()
        dbg_xn = nc.dram_tensor("dbg_xn", [P, P], f32,
                                kind="ExternalOutput").ap()
        dbg_ge = nc.dram_tensor("dbg_ge", [P, NBMAX * DZE], f32,
                                kind="ExternalOutput").ap()
        dbg_agg = nc.dram_tensor("dbg_agg", [P, P], f32,
                                 kind="ExternalOutput").ap()
        dbg_den = nc.dram_tensor("dbg_den", [P, 4], f32,
                                 kind="ExternalOutput").ap()
        dbg_msg = nc.dram_tensor("dbg_msg", [P, GB * P], f32,
                                 kind="ExternalOutput").ap()
    hsh = nc.dram_tensor("hsh", [IN_DIM, VSH], bf16)
    hfull = nc.dram_tensor("hfull", [NC, IN_DIM, VSH], bf16, addr_space="Shared")

    with tile.TileContext(nc) as tc, ExitStack() as ctx:
        cst = ctx.enter_context(tc.tile_pool(name="cst", bufs=1))
        sbh = ctx.enter_context(tc.tile_pool(name="sbh", bufs=3))   # hT chunks
        sbz = ctx.enter_context(tc.tile_pool(name="sbz", bufs=3))   # phase1 z out
        sbg = ctx.enter_context(tc.tile_pool(name="sbg", bufs=2))   # ge tiles
        sbo = ctx.enter_context(tc.tile_pool(name="sbo", bufs=2))   # ohS per tile
        sbw = ctx.enter_context(tc.tile_pool(name="sbw", bufs=4))   # small work
        sbm = ctx.enter_context(tc.tile_pool(name="sbm", bufs=3))   # msg / ohTg
        psZ = ctx.enter_context(tc.tile_pool(name="psZ", bufs=2, space="PSUM"))
        psO = ctx.enter_context(tc.tile_pool(name="psO", bufs=2, space="PSUM"))
        psB = ctx.enter_context(tc.tile_pool(name="psB", bufs=2, space="PSUM"))
        psR = ctx.enter_context(tc.tile_pool(name="psR", bufs=2, space="PSUM"))

        # ---- constants ----
        iotaFS = cst.tile([P, P], f32)
        nc.sync.dma_start(out=iotaFS[:], in_=iotd[:, :])
        iotaB = cst.tile([P, P], bf16)
        nc.vector.tensor_copy(out=iotaB[:], in_=iotaFS[:])
        eyeF = cst.tile([P, P], f32)
        nc.sync.dma_start(out=eyeF[:], in_=eyed[:, :])
        eyeB = cst.tile([P, P], bf16)
        nc.vector.tensor_copy(out=eyeB[:], in_=eyeF[:])
        E4S = cst.tile([4, P], f32)
        nc.sync.dma_start(out=E4S[:], in_=E4d[:, :])
        hwF = cst.tile([HC, 1], f32)
        nc.sync.dma_start(out=hwF[:], in_=hwd[:, :])
        hwS = cst.tile([HC, 1], bf16)
        nc.vector.tensor_copy(out=hwS[:], in_=hwF[:])
        WextF = cst.tile([IN_DIM, NLAYERS * (HC + 8)], f32)
        for li in range(NLAYERS):
            nc.sync.dma_start(out=WextF[:, li * (HC + 8):(li + 1) * (HC + 8)],
                              in_=Wextd[li, :, :])
        WextS = cst.tile([IN_DIM, NLAYERS * (HC + 8)], bf16)
        nc.vector.tensor_copy(out=WextS[:], in_=WextF[:])
        for li in range(NLAYERS):
            nc.sync.dma_start(out=WextF[:, li * (HC + 8):(li + 1) * (HC + 8)],
                              in_=Wextld[li, :, :])
        WextLoS = cst.tile([IN_DIM, NLAYERS * (HC + 8)], bf16)
        nc.vector.tensor_copy(out=WextLoS[:], in_=WextF[:])
        idxS = cst.tile([P, NBTOT * 8], i16)
        nc.sync.dma_start(out=idxS[:], in_=idxd[:, :])
        dcolF = cst.tile([P, NBTOT], f32)
        nc.sync.dma_start(out=dcolF[:], in_=dcold[:, :])
        tnsS = cst.tile([P, TILES], i32)
        nc.sync.dma_start(out=tnsS[:], in_=tnd[:, :])
        logitS = cst.tile([1, VSH], f32)
        zsbs = []
        for zi in range(3):
            zt = cst.tile([P, CHG * DZE], bf16, name=f"zsb{zi}")
            nc.vector.memset(zt[:], 0.0)
            zsbs.append(zt)

        Exp = mybir.ActivationFunctionType.Exp
        Copy = mybir.ActivationFunctionType.Copy

        for _rep in range(repeat):
         for li in range(NLAYERS):
            WextL = WextS[:, li * (HC + 8):(li + 1) * (HC + 8)]
            WextLo = WextLoS[:, li * (HC + 8):(li + 1) * (HC + 8)]
            ztab = ztabs[li % 2]
            # ================= Phase 1 =================
            for g0 in range(0, NCH, CHG):
                J = min(CHG, NCH - g0)
                n0 = g0 * P
                hT = sbh.tile([IN_DIM, CHG * P], bf16, tag="hT")
                hTlo = None
                if li == 0:
                    nc.sync.dma_start(out=hT[:, :J * P],
                                      in_=xTbd[:, n0:n0 + J * P])
                    hTlo = sbh.tile([IN_DIM, CHG * P], bf16, tag="hTlo")
                    nc.sync.dma_start(out=hTlo[:, :J * P],
                                      in_=xTld[:, n0:n0 + J * P])
                else:
                    done = 0
                    while done < J * P:
                        g = n0 + done
                        blk, col = g // VSH, g % VSH
                        w = min(J * P - done, VSH - col)
                        nc.sync.dma_start(out=hT[:, done:done + w],
                                          in_=hfull.ap()[blk, :, col:col + w])
                        done += w
                zps = psZ.tile([P, CHG * (HC + 8)], f32, space="PSUM", tag="zps")
                for j in range(J):
                    sl = zps[:, j * (HC + 8):(j + 1) * (HC + 8)]
                    nc.tensor.matmul(out=sl, lhsT=hT[:, j * P:(j + 1) * P],
                                     rhs=WextL, start=True, stop=False,
                                     skip_group_check=True)
                    nc.tensor.matmul(out=sl, lhsT=hT[:, j * P:(j + 1) * P],
                                     rhs=WextLo, start=False, stop=hTlo is None,
                                     skip_group_check=True)
                    if hTlo is not None:
                        nc.tensor.matmul(out=sl,
                                         lhsT=hTlo[:, j * P:(j + 1) * P],
                                         rhs=WextL, start=False, stop=True,
                                         skip_group_check=True)
                zsb = zsbs[(g0 // CHG) % 3]
                zpsR = zps.rearrange("p (j c) -> p j c", c=HC + 8)
                # z part -> bf16 cols 0..127 of each row
                zsbB = zsb.rearrange("p (j c) -> p j c", c=DZE)
                nc.scalar.activation(out=zsbB[:, :J, 0:HC],
                                     in_=zpsR[:, :J, 0:HC], func=Copy)
                # scores -> f32 at bytes 256..287 (f32 elements 64..71)
                zsbF = zsb.bitcast(f32).rearrange("p (j c) -> p j c",
                                                  c=DZE // 2)
                nc.scalar.activation(out=zsbF[:, :J, 64:72],
                                     in_=zpsR[:, :J, HC:HC + 8], func=Copy)
                # rows n0 + p*J + j  <->  sbuf partition p, cols j*DZE..
                dst_ap = AP(ztab, n0 * DZE, [[DZE, P * J], [1, DZE]])
                nc.sync.dma_start(out=dst_ap, in_=zsb[:, :J * DZE])

            # ================= Phase 2 =================
            for t in range(TILES):
                nbt = NBLO[t] + NBHI[t]
                if nbt == 0:
                    continue
                ngt = -(-nbt // GB)
                ge = sbg.tile([P, NBMAX, DZE], bf16, tag="ge")
                # gathers (lo from ztab[0:HSPLIT], hi from ztab[HSPLIT:])
                for sec, (nb, b0s) in enumerate([(NBLO[t], 0), (NBHI[t], NBLO[t])]):
                    if nb == 0:
                        continue
                    base = 0 if sec == 0 else HSPLIT
                    rows = HSPLIT if sec == 0 else Vp - HSPLIT
                    src_ap = AP(ztab, base * DZE, [[DZE, rows], [1, DZE]])
                    co = (NB0[t] + b0s) * 8
                    nc.gpsimd.dma_gather(
                        out_ap=ge[:, b0s:b0s + nb, :],
                        in_ap=src_ap,
                        idxs_ap=idxS[:, co:co + nb * 8],
                        num_idxs=nb * P, num_idxs_reg=nb * P,
                        elem_size=DZE, single_packet=False)
                # sdst rows (f32 bytes 272..287 = bf16 elems 136..143)
                sdstS = sbw.tile([P, 8], bf16, tag="sdstS")
                nc.gpsimd.indirect_dma_start(
                    out=sdstS[:], out_offset=None, in_=ztab.ap()[:, :],
                    in_offset=IndirectOffsetOnAxis(ap=tnsS[:, t:t + 1], axis=0),
                    element_offset=HC + 8)
                sdstF = sdstS.bitcast(f32)
                # --- sweep 1: oh, ohT, sde ---
                # AD bank: ONE accumulation group: aggT [:,0:128] + denTT
                # [:,128:132] (agg bb=0 starts it, den bb=last stops it).
                big = psB.tile([P, 512], f32, space="PSUM", tag="big")
                aggT = big[:, 0:P]
                denTT = big[:, P:P + 4]
                # SD bank: sequential producers, each consumed before the
                # next start pending-zeroes the bank.
                sdb = psR.tile([P, 512], f32, space="PSUM", tag="sdb")
                sdePS = sdb[:, 0:nbt * 4]
                ohS = sbo.tile([P, nbt * P], bf16, tag="ohS")
                for g in range(ngt):
                    nbl = min(GB, nbt - g * GB)
                    ohTps = psO.tile([P, GB * P], bf16, space="PSUM", tag="ohTps")
                    for b in range(nbl):
                        col = NB0[t] + g * GB + b
                        sl = (g * GB + b) * P
                        nc.vector.tensor_scalar(
                            out=ohS[:, sl:sl + P], in0=iotaB[:],
                            scalar1=dcolF[:, col:col + 1], scalar2=None,
                            op0=mybir.AluOpType.is_equal)
                        nc.tensor.transpose(out=ohTps[:, b * P:(b + 1) * P],
                                            in_=ohS[:, sl:sl + P],
                                            identity=eyeB[:])
                    ohTg = sbm.tile([P, GB * P], f32, tag="ohTg")
                    nc.scalar.activation(out=ohTg[:, :nbl * P],
                                         in_=ohTps[:, :nbl * P], func=Copy)
                    for b in range(nbl):
                        bb = g * GB + b
                        nc.tensor.matmul(out=sdb[:, bb * 4:(bb + 1) * 4],
                                         lhsT=ohTg[:, b * P:(b + 1) * P],
                                         rhs=sdstF, start=True, stop=True,
                                         skip_group_check=True)
                # --- tile-level score pipeline (f32) ---
                geF = ge.bitcast(f32)           # [128, NBMAX, 128] f32 view
                escb = sbw.tile([P, NBMAX * 4], f32, tag="escb")
                nc.vector.tensor_tensor(out=escb[:, :nbt * 4],
                                        in0=geF[:, :nbt, 64:68],
                                        in1=sdePS,
                                        op=mybir.AluOpType.add)
                esc2 = sbw.tile([P, NBMAX * 4], f32, tag="esc2")
                nc.vector.tensor_scalar(out=esc2[:, :nbt * 4],
                                        in0=escb[:, :nbt * 4], scalar1=NEG,
                                        scalar2=None, op0=mybir.AluOpType.mult)
                nc.vector.tensor_tensor(out=escb[:, :nbt * 4],
                                        in0=escb[:, :nbt * 4],
                                        in1=esc2[:, :nbt * 4],
                                        op=mybir.AluOpType.max)
                expf = sbw.tile([P, NBMAX * 4], bf16, tag="expf")
                nc.scalar.activation(out=expf[:, :nbt * 4],
                                     in_=escb[:, :nbt * 4], func=Exp)
                # --- sweep 2: msg, agg, den ---
                for g in range(ngt):
                    nbl = min(GB, nbt - g * GB)
                    msgS = sbm.tile([P, GB * P], bf16, tag="msgS")
                    z_ap = AP(ge.tensor, ge.offset + (g * GB) * DZE,
                              [ge.ap[0], [DZE, nbl], [1, HC]])
                    e_ap = AP(expf.tensor, expf.offset + (g * GB) * 4,
                              [expf.ap[0], [4, nbl], [0, COUT], [1, 4]])
                    nc.vector.tensor_tensor(out=msgS[:, :nbl * P], in0=z_ap,
                                            in1=e_ap, op=mybir.AluOpType.mult)
                    for b in range(nbl):
                        bb = g * GB + b
                        first = bb == 0
                        last = bb == nbt - 1
                        # agg+den share ONE psum accumulation group: agg's
                        # first matmul starts it, den's last matmul stops it.
                        nc.tensor.matmul(out=aggT, lhsT=msgS[:, b * P:(b + 1) * P],
                                         rhs=ohS[:, bb * P:(bb + 1) * P],
                                         start=first, stop=False,
                                         skip_group_check=True)
                        nc.tensor.matmul(out=denTT,
                                         lhsT=ohS[:, bb * P:(bb + 1) * P],
                                         rhs=expf[:, bb * 4:(bb + 1) * 4],
                                         start=False, stop=last,
                                         skip_group_check=True)
                # --- finalize ---
                rexPS = sdb[:, 256:256 + P]
                dsbT = sdb[0:4, 384:512]
                lg = sdb[0:1, 0:P]
                dsb = sbw.tile([P, 4], f32, tag="dsb")
                nc.vector.tensor_scalar(out=dsb[:], in0=denTT, scalar1=1e-9,
                                        scalar2=None, op0=mybir.AluOpType.add)
                nc.vector.reciprocal(out=dsb[:], in_=dsb[:])
                nc.tensor.transpose(out=dsbT, in_=dsb[:], identity=eyeF[:])
                dsbT2 = sbw.tile([4, P], f32, tag="dsbT2")
                nc.scalar.activation(out=dsbT2[:], in_=dsbT, func=Copy)
                nc.tensor.matmul(out=rexPS, lhsT=E4S[:], rhs=dsbT2[:],
                                 start=True, stop=True)
                rexS = sbw.tile([P, P], f32, tag="rexS")
                nc.scalar.activation(out=rexS[:], in_=rexPS, func=Copy)
                # last layer keeps f32 through the head dot (bf16 h there
                # costs ~0.4% directly on the logits)
                hdt = bf16 if li < NLAYERS - 1 else f32
                xn = sbw.tile([P, P], hdt, tag="xn")
                nc.vector.tensor_tensor(out=xn[:], in0=aggT, in1=rexS[:],
                                        op=mybir.AluOpType.mult)
                # ELU
                tmin = sbw.tile([P, P], hdt, tag="tmin")
                nc.vector.tensor_scalar(out=tmin[:], in0=xn[:], scalar1=0.0,
                                        scalar2=None, op0=mybir.AluOpType.min)
                texp = sbw.tile([P, P], hdt, tag="texp")
                nc.scalar.activation(out=texp[:], in_=tmin[:], func=Exp)
                trel = sbw.tile([P, P], hdt, tag="trel")
                nc.vector.tensor_scalar(out=trel[:], in0=xn[:], scalar1=0.0,
                                        scalar2=None, op0=mybir.AluOpType.max)
                hsum = sbw.tile([P, P], hdt, tag="hsum")
                nc.vector.tensor_tensor(out=hsum[:], in0=trel[:], in1=texp[:],
                                        op=mybir.AluOpType.add)
                hb16 = sbw.tile([P, P], hdt, tag="hb16")
                nc.vector.tensor_scalar(out=hb16[:], in0=hsum[:], scalar1=-1.0,
                                        scalar2=None, op0=mybir.AluOpType.add)
                if debug_tile is not None and li == 0 and t == debug_tile:
                    dbf = sbw.tile([P, NBMAX * DZE], f32, tag="dbf", bufs=1)
                    nc.vector.tensor_copy(out=dbf[:, :nbt * P], in_=ohS[:])
                    nc.sync.dma_start(out=dbg_oh[:, :nbt * P],
                                      in_=dbf[:, :nbt * P])
                    nc.vector.tensor_copy(out=dbf[:, :4], in_=sdstF)
                    nc.sync.dma_start(out=dbg_sd[:, :4], in_=dbf[:, :4])
                    nc.vector.tensor_copy(out=dbf[:, :nbt * 4],
                                          in_=escb[:, :nbt * 4])
                    nc.sync.dma_start(out=dbg_esc[:, :nbt * 4],
                                      in_=dbf[:, :nbt * 4])
                    nc.vector.tensor_copy(out=dbf[:, :nbt * 4],
                                          in_=expf[:, :nbt * 4])
                    nc.sync.dma_start(out=dbg_exp[:, :nbt * 4],
                                      in_=dbf[:, :nbt * 4])
                    nc.vector.tensor_copy(out=dbf[:, :P], in_=xn[:])
                    nc.sync.dma_start(out=dbg_xn[:, :P], in_=dbf[:, :P])
                    nc.vector.tensor_copy(out=dbf[:, :nbt * DZE],
                                          in_=ge[:, :nbt, :])
                    nc.sync.dma_start(out=dbg_ge[:, :nbt * DZE],
                                      in_=dbf[:, :nbt * DZE])
                    nc.vector.tensor_copy(out=dbf[:, :P], in_=aggT)
                    nc.sync.dma_start(out=dbg_agg[:, :], in_=dbf[:, :P])
                    nc.vector.tensor_copy(out=dbf[:, :4], in_=denTT)
                    nc.sync.dma_start(out=dbg_den[:, :], in_=dbf[:, :4])
                if li < NLAYERS - 1:
                    nc.sync.dma_start(out=hsh.ap()[:, t * P:(t + 1) * P],
                                      in_=hb16[:])
                else:
                    nc.tensor.matmul(out=lg, lhsT=hwF[:], rhs=hb16[:],
                                     start=True, stop=True)
                    nc.vector.tensor_scalar(out=logitS[:, t * P:(t + 1) * P],
                                            in0=lg, scalar1=hb, scalar2=None,
                                            op0=mybir.AluOpType.add)
            if li < NLAYERS - 1:
                nc.gpsimd.collective_compute(
                    "AllGather", mybir.AluOpType.bypass,
                    replica_groups=[list(range(NC))],
                    ins=[hsh.ap()[:, :]], outs=[hfull.ap()[:, :, :]])
        nc.sync.dma_start(out=out[None, :], in_=logitS[:])
    nc.compile()
    return nc


# ======================= runner =======================

import time
import numpy as np
import jax
from jax.sharding import Mesh, PartitionSpec
from jax.experimental.shard_map import shard_map

import concourse.mybir as mybir
from concourse import bass2jax
from concourse.bass2jax import _bass_exec_p, install_neuronx_cc_hook, partition_id_tensor


class SpmdRunner:
    def __init__(self, nc, n_cores: int):
        install_neuronx_cc_hook()
        assert nc.dbg_addr is None or not nc.dbg_callbacks
        self.nc = nc
        self.n_cores = n_cores
        partition_name = nc.partition_id_tensor.name if nc.partition_id_tensor else None

        in_names, out_names, out_avals, zero_outs = [], [], [], []
        for alloc in nc.m.functions[0].allocations:
            if not isinstance(alloc, mybir.MemoryLocationSet):
                continue
            name = alloc.memorylocations[0].name
            if alloc.kind == "ExternalInput":
                if name != partition_name and name != (nc.dbg_addr.name if nc.dbg_addr else None):
                    in_names.append(name)
            elif alloc.kind == "ExternalOutput":
                out_names.append(name)
                shape = tuple(alloc.tensor_shape)
                dtype = mybir.dt.np(alloc.dtype)
                out_avals.append(jax.core.ShapedArray(shape, dtype))
                zero_outs.append(np.zeros(shape, dtype))
        self.in_names, self.out_names = in_names, out_names
        self.out_avals, self.zero_outs = out_avals, zero_outs
        n_params = len(in_names)
        self.n_params = n_params
        n_outs = len(out_avals)

        all_in_names = list(in_names) + list(out_names)
        if nc.dbg_addr is not None:
            all_in_names.append(nc.dbg_addr.name)
        if partition_name is not None:
            all_in_names.append(partition_name)

        dbg_name = nc.dbg_addr.name if nc.dbg_addr is not None else None

        def _body(*args):
            operands = list(args)
            if dbg_name is not None:
                operands.append(np.zeros((1, 2), np.uint32))
            if partition_name is not None:
                operands.append(partition_id_tensor())
            outs = _bass_exec_p.bind(
                *operands,
                out_avals=tuple(out_avals),
                in_names=tuple(all_in_names),
                out_names=tuple(out_names),
                lowering_input_output_aliases=(),
                sim_require_finite=False,
                sim_require_nnan=False,
                nc=nc,
            )
            return tuple(outs)

        devices = jax.devices()[:n_cores]
        assert len(devices) == n_cores
        self.mesh = Mesh(np.asarray(devices), ("core",))
        in_specs = (PartitionSpec("core"),) * (n_params + n_outs)
        out_specs = (PartitionSpec("core"),) * n_outs
        self.donate = tuple(range(n_params, n_params + n_outs))
        self.fn = jax.jit(
            shard_map(_body, mesh=self.mesh, in_specs=in_specs,
                      out_specs=out_specs, check_rep=False),
            donate_argnums=self.donate, keep_unused=True,
        )
        self.concat_in = None

    def load_inputs(self, in_maps):
        """Concat per-core inputs and push to devices once."""
        assert len(in_maps) == self.n_cores
        per_core = [[np.asarray(m[name]) for name in self.in_names] for m in in_maps]
        concat = [np.concatenate([per_core[c][i] for c in range(self.n_cores)], axis=0)
                  for i in range(self.n_params)]
        sh = jax.sharding.NamedSharding(self.mesh, PartitionSpec("core"))
        self.concat_in = [jax.device_put(a, sh) for a in concat]

    def _zeros(self):
        sh = jax.sharding.NamedSharding(self.mesh, PartitionSpec("core"))
        return [jax.device_put(np.zeros((self.n_cores * z.shape[0], *z.shape[1:]), z.dtype), sh)
                for z in self.zero_outs]

    def run(self):
        outs = self.fn(*self.concat_in, *self._zeros())
        jax.block_until_ready(outs)
        return [
            {name: np.asarray(outs[i]).reshape(self.n_cores, *self.out_avals[i].shape)[c]
             for i, name in enumerate(self.out_names)}
            for c in range(self.n_cores)
        ]

    def time(self, iters=8, warmup=2):
        """Per-call wall time (s) for the jitted executable, zeros pre-staged."""
        zs = [self._zeros() for _ in range(iters + warmup)]
        for i in range(warmup):
            jax.block_until_ready(self.fn(*self.concat_in, *zs[i]))
        ts = []
        for i in range(iters):
            t0 = time.perf_counter()
            jax.block_until_ready(self.fn(*self.concat_in, *zs[warmup + i]))
            ts.append(time.perf_counter() - t0)
        return min(ts), ts



# ======================= driver =======================
import time as _time
import jax as _jax

_CACHE = {}
LAST_EXEC_NS = None


def _floor_nc(ncores):
    """Tiny kernel to estimate the per-call dispatch floor."""
    nc = bacc.Bacc("TRN2", target_bir_lowering=False, debug=False, num_devices=ncores)
    a = nc.dram_tensor("a", [P, 64], mybir.dt.float32, kind="ExternalInput").ap()
    b = nc.dram_tensor("b", [P, 64], mybir.dt.float32, kind="ExternalOutput").ap()
    with tile.TileContext(nc) as tc, ExitStack() as ctx:
        sb = ctx.enter_context(tc.tile_pool(name="sb", bufs=2))
        t = sb.tile([P, 64], mybir.dt.float32)
        nc.sync.dma_start(out=t[:], in_=a[:, :])
        nc.sync.dma_start(out=b[:, :], in_=t[:])
    nc.compile()
    return nc


def kernel(x, edge_index, Ws, a_src, a_dst, head_w, head_b, hsplit=32768,
           tiles_per_core=None, repeat=1):
    NC = 8
    V = int(np.asarray(x).shape[0])
    if tiles_per_core is None:
        tiles_per_core = (V + NC * P - 1) // (NC * P)
    cfg = make_cfg(V, NC, tiles_per_core, hsplit)
    in_maps, meta = host_prep(cfg, x, edge_index, Ws, a_src, a_dst,
                              head_w, head_b)
    key = (V, tuple(meta["NBLO"]), tuple(meta["NBHI"]))
    if key not in _CACHE:
        nc = build_nc(cfg, meta, repeat=repeat)
        r = SpmdRunner(nc, NC)
        _CACHE[key] = r
    r = _CACHE[key]
    r.load_inputs(in_maps)
    res = r.run()
    out = np.concatenate([res[c]["out"] for c in range(NC)])[:V]
    return out.astype(np.float32)


def measure(iters=48):
    """Estimate HW exec ns via interleaved kernel/floor timing."""
    global LAST_EXEC_NS
    assert _CACHE, "call kernel() first"
    r = next(iter(_CACHE.values()))
    fnc = _floor_nc(r.n_cores)
    fr = SpmdRunner(fnc, r.n_cores)
    fr.load_inputs([{"a": np.zeros((P, 64), np.float32)}] * r.n_cores)
    fr.run()
    r.run()
    diffs, ks, fs = [], [], []
    for _ in range(iters):
        z = r._zeros()
        t0 = _time.perf_counter()
        _jax.block_until_ready(r.fn(*r.concat_in, *z))
        tk = _time.perf_counter() - t0
        zf = fr._zeros()
        t0 = _time.perf_counter()
        _jax.block_until_ready(fr.fn(*fr.concat_in, *zf))
        tf = _time.perf_counter() - t0
        ks.append(tk); fs.append(tf); diffs.append(tk - tf)
    diffs.sort()
    med = diffs[len(diffs) // 2]
    LAST_EXEC_NS = int(max(0.0, med) * 1e9)
    return LAST_EXEC_NS, sorted(ks)[len(ks)//2], sorted(fs)[len(fs)//2]


def gat_reference_np(x, edge_index, Ws, a_src, a_dst, head_w, head_b):
    V = x.shape[0]
    src = np.asarray(edge_index[0]); dst = np.asarray(edge_index[1])
    h = np.asarray(x, np.float64)
    for li in range(len(Ws)):
        z = (h @ np.asarray(Ws[li], np.float64).T).reshape(V, HEADS, COUT)
        ss = np.einsum("vhc,hc->vh", z, np.asarray(a_src[li], np.float64))
        sd = np.einsum("vhc,hc->vh", z, np.asarray(a_dst[li], np.float64))
        e = ss[src] + sd[dst]
        e = np.where(e > 0, e, NEG * e)
        m = np.full((V, HEADS), -np.inf); np.maximum.at(m, dst, e)
        m = np.maximum(m, -1e9)
        ex = np.exp(e - m[dst])
        den = np.zeros((V, HEADS)); np.add.at(den, dst, ex)
        alpha = ex / (den[dst] + 1e-9)
        msg = z[src] * alpha[:, :, None]
        agg = np.zeros((V, HEADS, COUT)); np.add.at(agg, dst, msg)
        h = np.where(agg > 0, agg, np.expm1(agg)).reshape(V, HC)
    return (h @ np.asarray(head_w, np.float64).T + np.asarray(head_b)).reshape(V)
